# revision 4
# baseline (speedup 1.0000x reference)
"""Phase multi-head attention kernel for Trainium2 (Bass/Tile), 8-core SPMD.

Math (per batch b, head h, with state s = sr + i*si reshaped to (S, HD)):
    q = s * e^{i*q_rot},  k = s * e^{i*k_rot},  v = s * e^{i*v_rot}
    logits[s,t] = Re(q_s . conj(k_t)) = Re(s_s . conj(s_t) e^{i(q_rot-k_rot)})
Only K needs the (q_rot - k_rot) rotation:
    logits = [sr|si] @ (Mt @ [srT;siT]),   Mt = [[C,S],[-S,C]], phi = q_rot-k_rot
The softmax scale cancels (1/sqrt(HD) * 8.0 with HD=64), so attn = softmax
over t<=s of raw logits.  The V rotation is applied after the attention
product: out_r = cos(v)*U1 - sin(v)*U2, out_i = sin(v)*U1 + cos(v)*U2 where
[U1|U2] = attn @ [sr|si].

Sharding: head-parallel, core c owns head c and both batches (2 pairs/core).

Device layout per pair:
  nat  (128, 16, 128) f32   chunk n holds rows s=n*128+p, cols [sr|si]
  sT   (128, 2048)    f32   d-stack major ([srT;siT]), via 16 PE transposes
  kT   (128, 2048)    f32   Mt @ sT via 4 fp32r matmuls
  natb (128, 16, 130) bf16  [sr|si|1|pad] per chunk - PV rhs + denominator col
  For each 512-wide query block j: logitsT (t-chunk, sq) fp32r matmuls ->
  exp (ScalarE, bf16 out) -> causal mask on the diagonal sub-chunk
  (affine_select) -> PV accumulation psum[sq,129] over t-chunks (bf16),
  col 128 = softmax denominator.  Then reciprocal + scale + V-rotation.
"""

import numpy as np

import concourse.bass as bass
import concourse.bacc as bacc
import concourse.mybir as mybir
import concourse.tile as tile
from concourse.masks import make_identity
from concourse.bass_utils import run_bass_kernel_spmd

B, S, D = 2, 2048, 512
H, HD = 8, 64
P = 128
NCHUNK = S // P      # 16 seq chunks of 128
NBLK = 4             # query blocks of 512
BLKW = 512
NATW = 130           # [sr(64) | si(64) | ones(1) | pad(1)]

f32 = mybir.dt.float32
f32r = mybir.dt.float32r
bf16 = mybir.dt.bfloat16
EXP = mybir.ActivationFunctionType.Exp


def build_kernel():
    nc = bacc.Bacc("TRN2", target_bir_lowering=False)

    nat_d = [nc.dram_tensor(f"nat{p}", (P, NCHUNK, P), f32, kind="ExternalInput")
             for p in range(B)]
    mtT_d = nc.dram_tensor("mtT", (P, P), f32, kind="ExternalInput")
    cosv_d = nc.dram_tensor("cosv", (1, NCHUNK * HD), f32, kind="ExternalInput")
    sinv_d = nc.dram_tensor("sinv", (1, NCHUNK * HD), f32, kind="ExternalInput")
    out_d = [nc.dram_tensor(f"out{p}", (P, NCHUNK, P), f32, kind="ExternalOutput")
             for p in range(B)]

    with tile.TileContext(nc) as tc:
        with (
            tc.tile_pool(name="persist", bufs=1) as persist,
            tc.tile_pool(name="work", bufs=4) as work,
            tc.tile_pool(name="pwork", bufs=3, space="PSUM") as pwork,
            tc.tile_pool(name="pout", bufs=1, space="PSUM") as pout,
        ):
            ident = persist.tile([P, P], f32, tag="ident")
            make_identity(nc, ident)
            mtT_f = persist.tile([P, P], f32, tag="mtT_f")
            nc.sync.dma_start(out=mtT_f, in_=mtT_d[:, :])
            mtT = persist.tile([P, P], f32r, tag="mtT")
            nc.vector.tensor_copy(out=mtT, in_=mtT_f)
            cosv = persist.tile([P, NCHUNK * HD], f32, tag="cosv")
            nc.sync.dma_start(out=cosv, in_=cosv_d[:, :].to_broadcast((P, NCHUNK * HD)))
            sinv = persist.tile([P, NCHUNK * HD], f32, tag="sinv")
            nc.sync.dma_start(out=sinv, in_=sinv_d[:, :].to_broadcast((P, NCHUNK * HD)))

            nats, natbs, sTs, kTs = [], [], [], []
            for p in range(B):
                natf = persist.tile([P, NCHUNK, P], f32, tag=f"nat{p}")
                nc.sync.dma_start(out=natf, in_=nat_d[p][:, :, :])
                natb = persist.tile([P, NCHUNK, NATW], bf16, tag=f"natb{p}")
                nc.gpsimd.tensor_copy(out=natb[:, :, 0:P], in_=natf)
                nc.vector.memset(natb[:, :, P:P + 1], 1.0)

                sT = persist.tile([P, S], f32r, tag=f"sT{p}")
                for g in range(4):
                    ps = pwork.tile([P, BLKW], f32, tag="work")
                    for cc in range(4):
                        nc.tensor.transpose(ps[:, cc * P:(cc + 1) * P],
                                            natf[:, g * 4 + cc, :], ident)
                    nc.vector.tensor_copy(out=sT[:, g * BLKW:(g + 1) * BLKW], in_=ps)

                kT = persist.tile([P, S], f32r, tag=f"kT{p}")
                for g in range(4):
                    ps = pwork.tile([P, BLKW], f32, tag="work")
                    nc.tensor.matmul(ps, lhsT=mtT,
                                     rhs=sT[:, g * BLKW:(g + 1) * BLKW],
                                     start=True, stop=True)
                    nc.vector.tensor_copy(out=kT[:, g * BLKW:(g + 1) * BLKW], in_=ps)

                nats.append(natf)
                natbs.append(natb)
                sTs.append(sT)
                kTs.append(kT)

            out_alls = []
            for p in range(B):
                out_all = persist.tile([P, NCHUNK, P], f32, tag=f"outall{p}")
                for j in range(NBLK):
                    po = pout.tile([P, 4, BLKW], f32, tag="pout")
                    for c in range(4 * j + 4):
                        off = (c - 4 * j) * P if c >= 4 * j else 0
                        qkoff = off if off in (128, 256) else 0
                        psl = pwork.tile([P, BLKW], f32, tag="work")
                        nc.tensor.matmul(
                            psl[:, qkoff:],
                            lhsT=kTs[p][:, c * P:(c + 1) * P],
                            rhs=sTs[p][:, j * BLKW + qkoff:(j + 1) * BLKW],
                            start=True, stop=True)
                        ex = work.tile([P, BLKW], bf16, tag="ex")
                        nc.scalar.activation(out=ex[:, off:], in_=psl[:, off:], func=EXP)
                        if c >= 4 * j:
                            k0 = c - 4 * j
                            # keep where sq_local >= t_local (causal diagonal)
                            nc.gpsimd.affine_select(
                                out=ex[:, k0 * P:(k0 + 1) * P],
                                in_=ex[:, k0 * P:(k0 + 1) * P],
                                compare_op=mybir.AluOpType.is_ge, fill=0.0,
                                base=0, pattern=[[1, P]], channel_multiplier=-1)
                        for k in range(4):
                            if 4 * j + k >= c:
                                nc.tensor.matmul(
                                    po[:, k, 0:P + 1],
                                    lhsT=ex[:, k * P:(k + 1) * P],
                                    rhs=natbs[p][:, c, 0:P + 1],
                                    start=(c == 0), stop=(c == 4 * j + k))
                    rec = work.tile([P, 4], f32, tag="rec")
                    nc.vector.reciprocal(out=rec, in_=po[:, :, P])
                    for k in range(4):
                        nc.vector.tensor_scalar_mul(
                            out=out_all[:, 4 * j + k, :],
                            in0=po[:, k, 0:P], scalar1=rec[:, k:k + 1])
                out_alls.append(out_all)

            cosv_v = cosv.rearrange("p (n d) -> p n d", d=HD)
            sinv_v = sinv.rearrange("p (n d) -> p n d", d=HD)
            for p in range(B):
                A = out_alls[p][:, :, 0:HD]
                Bv = out_alls[p][:, :, HD:2 * HD]
                of = persist.tile([P, NCHUNK, P], f32, tag=f"outfin{p}")
                t1 = work.tile([P, NCHUNK, HD], f32, tag="t1")
                t2 = work.tile([P, NCHUNK, HD], f32, tag="t2")
                t3 = work.tile([P, NCHUNK, HD], f32, tag="t3")
                t4 = work.tile([P, NCHUNK, HD], f32, tag="t4")
                nc.vector.tensor_mul(t1, A, cosv_v)
                nc.gpsimd.tensor_mul(t2, Bv, sinv_v)
                nc.vector.tensor_sub(of[:, :, 0:HD], t1, t2)
                nc.gpsimd.tensor_mul(t3, A, sinv_v)
                nc.vector.tensor_mul(t4, Bv, cosv_v)
                nc.gpsimd.tensor_add(of[:, :, HD:2 * HD], t3, t4)
                nc.sync.dma_start(out=out_d[p][:, :, :], in_=of)

    nc.compile()
    return nc


def make_in_maps(state_real, state_imag, q_rot, k_rot, v_rot):
    """Per-core input dicts: core c gets head c, both batches."""
    in_maps = []
    for c in range(H):
        phi = (q_rot[c] - k_rot[c]).astype(np.float32)
        Cp, Sp = np.cos(phi), np.sin(phi)
        mtT = np.block([[np.diag(Cp), np.diag(-Sp)],
                        [np.diag(Sp), np.diag(Cp)]]).astype(np.float32)
        cv = np.tile(np.cos(v_rot[c]).astype(np.float32), NCHUNK)[None, :]
        sv = np.tile(np.sin(v_rot[c]).astype(np.float32), NCHUNK)[None, :]
        m = {"mtT": np.ascontiguousarray(mtT),
             "cosv": np.ascontiguousarray(cv),
             "sinv": np.ascontiguousarray(sv)}
        for p in range(B):
            nat = np.concatenate(
                [state_real[p, :, c * HD:(c + 1) * HD],
                 state_imag[p, :, c * HD:(c + 1) * HD]], axis=1)  # (S, 128)
            natp = nat.reshape(NCHUNK, P, P).transpose(1, 0, 2)  # (128, 16, 128)
            m[f"nat{p}"] = np.ascontiguousarray(natp.astype(np.float32))
        in_maps.append(m)
    return in_maps


def assemble_output(results):
    """results: list of 8 dicts with out0/out1 (128, 16, 128) f32."""
    out = np.zeros((B, S, D), dtype=np.complex64)
    for c in range(H):
        for p in range(B):
            o = results[c][f"out{p}"]                      # (128, 16, 128)
            om = o.transpose(1, 0, 2).reshape(S, P)        # (2048, 128)
            out[p, :, c * HD:(c + 1) * HD] = om[:, :HD] + 1j * om[:, HD:]
    return out


_NC_CACHE = []


def kernel(state_real, state_imag, q_rot, k_rot, v_rot):
    state_real = np.asarray(state_real, dtype=np.float32)
    state_imag = np.asarray(state_imag, dtype=np.float32)
    q_rot = np.asarray(q_rot, dtype=np.float32)
    k_rot = np.asarray(k_rot, dtype=np.float32)
    v_rot = np.asarray(v_rot, dtype=np.float32)

    if not _NC_CACHE:
        _NC_CACHE.append(build_kernel())
    nc = _NC_CACHE[0]

    in_maps = make_in_maps(state_real, state_imag, q_rot, k_rot, v_rot)
    res = run_bass_kernel_spmd(nc, in_maps, core_ids=list(range(H)))
    return assemble_output(res.results)


if __name__ == "__main__":
    rng = np.random.default_rng(0)
    inputs = {
        "state_real": rng.standard_normal((B, S, D), dtype=np.float32),
        "state_imag": rng.standard_normal((B, S, D), dtype=np.float32),
        "q_rot": rng.uniform(-np.pi, np.pi, (H, HD)).astype(np.float32),
        "k_rot": rng.uniform(-np.pi, np.pi, (H, HD)).astype(np.float32),
        "v_rot": rng.uniform(-np.pi, np.pi, (H, HD)).astype(np.float32),
    }
    out = kernel(**inputs)
    print("ran:", out.shape, out.dtype)
